# revision 26
# baseline (speedup 1.0000x reference)
"""Bi-directional minGRU Trainium2 kernel (v2).

Full-input contract: kernel(**inputs) takes the unsharded numpy inputs from
reference.setup_inputs() and returns the full (B, L, 1) float32 output.

Sharding: data-parallel over batch B=32 across 8 NeuronCores (4 sequences per
core), parameters replicated. Per core, feature-on-partition / time-on-free:

  te1    : w1*t+b1 on PE (K=2 matmul), Relu drain on ACT -> rr rows 0..63
  rr     : [te1(64) ; x(2) ; ones(1)] -> 67 partitions (ones row folds the
           proj bias into the matmul); ones row persistent per buffer
  gates  : rr @ [wz|wh] on PE (K=67, 512-col chunks, 2048-col PSUM tiles)
  z,h~   : single 2048-col ACT drains (Sigmoid/Tanh + bias)
  b=z*h~ : Pool tensor_tensor (runs concurrently with DVE); a few units on
           DVE tensor_tensor (bf16 2x mode) to balance
  a=1-z  : DVE tensor_scalar (bf16 4x mode)
  scan   : DVE tensor_tensor_scan state = a*state + b, shifted one step
           (reference stores pre-update state); zero edge columns are
           persistent per hv buffer; backward runs through negative-stride APs
  head   : [h_fwd;h_bwd;t_enc] @ gh_w1 (5 k-tiles) on PE, Relu+bias ACT drain
           -> bf16 rt, DMA'd to HBM; the final @gh_w2 + b2 runs on host.
"""

import time

import numpy as np
import ml_dtypes

import concourse.bass as bass
import concourse.mybir as mybir
import concourse.tile as tile
from concourse.vector_clock import ScopedClock, VectorClock
from concourse.bass_utils import run_bass_kernel_spmd

# ---------------------------------------------------------------------------
# Workaround for a walrus codegen limit in this toolchain: the TileContext
# tail drain carries one sync-wait per live proc sem, but this walrus build
# rejects >2 sync waits on a Drain (CTRL_NO_STRUCT template). Re-emit the tail
# with the waits split across single-wait NOPs on the sync engine (same-engine
# program order preserves the semantics), followed by a wait-free drain.
# ---------------------------------------------------------------------------


def _patched_drain_and_barrier(self, tick_clock, wait_clock):
    nc = self.nc
    vals = list(tick_clock.global_clock)
    n = len(vals)
    for i, v in enumerate(vals):
        if v > 0:
            partial = [0] * n
            partial[i] = v
            nop = nc.sync.nop()
            wait_clock.add_sem_waits(nop.ins, ScopedClock({None: VectorClock(partial)}))
    nc.sync.drain()
    nc.all_engine_barrier()
    assert self.sems is not None
    popped = nc._tile_sem_poison_stack.pop()
    assert popped is self._sem_poison
    nc.clear_and_free_semaphores(list(self.sems.allocated().values()))
    nc.all_engine_barrier()


tile.TileContext._drain_and_barrier = _patched_drain_and_barrier


def _spill_excess_waits(nc, maxw=1):
    """Split instructions carrying more than `maxw` sem waits: the excess
    waits move onto NoOps inserted just before, on the same engine (same-
    engine program order keeps the semantics identical)."""
    for bb in nc.m.functions[0].blocks:
        new = []
        for inst in bb.instructions:
            si = inst.sync_info
            if si is not None and si.on_wait is not None and len(si.on_wait) > maxw:
                waits = list(si.on_wait)
                excess, keep = waits[:-maxw], waits[-maxw:]
                for j, w in enumerate(excess):
                    nop = mybir.InstNoOp(
                        name=f"{inst.name}_ws{j}",
                        engine=inst.engine,
                        ins=[],
                        outs=[],
                        sync_info=mybir.SyncInfo(on_wait=[w], on_update=[]),
                    )
                    nc.register_instruction(nop)
                    new.append(nop)
                si.on_wait = keep
            new.append(inst)
        if len(new) != len(bb.instructions):
            _replace_block_instructions(bb, new)


def _replace_block_instructions(bb, new):
    try:
        bb.instructions = new
    except Exception:
        while len(bb.instructions):
            bb.instructions.pop()
        for inst in new:
            bb.add_instruction(inst)

# ---------------------------------------------------------------------------

B, L, H, TE = 32, 2048, 256, 64
NCORES = 8
BS = B // NCORES           # sequences per core
HH = 128                   # gauss head hidden
IN_AUG = TE + 2 + 1        # rr rows: te1(64) + x(2) + ones(1)
F32 = mybir.dt.float32

DT = mybir.dt.bfloat16     # matmul/activation storage dtype
NP_DT = ml_dtypes.bfloat16

FCH = 512                  # matmul chunk (one PSUM bank fp32)
PSC = 2048                 # PSUM tile columns (4 banks)


def _rev(t, cols, ncols):
    """Reversed-free-dim view of tile AP t over columns [cols, cols+ncols)."""
    return bass.AP(
        tensor=t.tensor,
        offset=t.offset + cols + ncols - 1,
        ap=[list(t.ap[0]), [-1, ncols]],
    )


def _build_nc(bs=BS, repeats=1, b_dve_idx=tuple(range(16)), hp_bufs=2, mp_bufs=3,
              ps_bufs=2):
    nc = bass.Bass("TRN2", target_bir_lowering=False, debug=False, num_devices=NCORES)

    d_xT = nc.dram_tensor("xT", [bs, 2, L], DT, kind="ExternalInput")
    d_t = nc.dram_tensor("t", [bs, L], DT, kind="ExternalInput")
    d_tw = nc.dram_tensor("tw", [2, TE], DT, kind="ExternalInput")
    # bundled weights: gw = [wzf|whf|wzb|whb] (67,1024); g1p = g1 row-blocks
    # side by side (128,512); bias cols = [bzf0,bzf1,bhf0,bhf1,bzb0,bzb1,
    # bhb0,bhb1,gb1] (128,9)
    d_gw = nc.dram_tensor("gw", [IN_AUG, 4 * H], DT, kind="ExternalInput")
    d_bias = nc.dram_tensor("bias", [128, 9], F32, kind="ExternalInput")
    d_g1 = nc.dram_tensor("g1", [128, 4 * HH + 1], DT, kind="ExternalInput")
    d_g1te = nc.dram_tensor("g1te", [IN_AUG, HH], DT, kind="ExternalInput")
    d_out = nc.dram_tensor("out", [bs, L], F32, kind="ExternalOutput")

    ALU = mybir.AluOpType
    AF = mybir.ActivationFunctionType

    with tile.TileContext(nc) as tc:
        with (
            tc.tile_pool(name="wpool", bufs=1) as wp,
            tc.tile_pool(name="mpool", bufs=mp_bufs) as mp,
            tc.tile_pool(name="hpool", bufs=hp_bufs) as hp,
            tc.tile_pool(name="psum", bufs=ps_bufs, space="PSUM") as pp,
        ):
            # ---- replicated weights, loaded once ----
            # sync queue is reserved for the startup critical path (tw + the
            # first sequence's inputs); bulk weights go on gpsimd/scalar.
            _eng = [nc.gpsimd, nc.scalar]
            _ei = [0]

            def wload(shape, dtype, tag, src_ap, eng=None):
                t_ = wp.tile(shape, dtype, tag=tag, name=tag)
                (eng or _eng[_ei[0] % len(_eng)]).dma_start(out=t_, in_=src_ap)
                _ei[0] += 1
                return t_

            # gw split across queues: a single DMA transfer runs on one hw
            # engine (~22GB/s), so large loads are striped across queues
            s_tw = wload([2, TE], DT, "tw", d_tw[:, :], eng=nc.sync)
            s_gw = wp.tile([IN_AUG, 4 * H], DT, tag="gw", name="gw")
            nc.sync.dma_start(out=s_gw[:, 0:512], in_=d_gw[:, 0:512])
            nc.gpsimd.dma_start(out=s_gw[:, 512:1024], in_=d_gw[:, 512:1024])
            s_bias = wload([128, 9], F32, "bias", d_bias[:, :], eng=nc.scalar)
            s_g1p = wload([128, 4 * HH + 1], DT, "g1p", d_g1[:, :], eng=nc.scalar)
            s_g1te = wload([IN_AUG, HH], DT, "g1te", d_g1te[:, :], eng=nc.scalar)
            s_g2 = s_g1p[:, 4 * HH:4 * HH + 1]
            def _gw_slices(base):
                return [s_gw[:, base + 128 * p:base + 128 * (p + 1)]
                        for p in range(2)]

            s_wz = {"f": _gw_slices(0), "b": _gw_slices(2 * H)}
            s_wh = {"f": _gw_slices(H), "b": _gw_slices(3 * H)}
            s_bz = {d: [s_bias[:, c:c + 1] for c in ((0, 1) if d == "f" else (4, 5))]
                    for d in "fb"}
            s_bh = {d: [s_bias[:, c:c + 1] for c in ((2, 3) if d == "f" else (6, 7))]
                    for d in "fb"}
            s_gb1 = s_bias[:, 8:9]
            s_g1 = [s_g1p[:, 128 * j:128 * (j + 1)] for j in range(4)]

            # pre-allocate rr and hv buffers; persistent rows/columns (ones
            # row, scan zero edge) are memset once upfront on the idle DVE
            rr_tiles = [mp.tile([IN_AUG, L], DT, tag="rr", name="rr", bufs=2)
                        for _ in range(2)]
            for t_ in rr_tiles:
                nc.vector.memset(t_[TE:TE + 3, :], 1.0)
            t2_tiles = [mp.tile([2, L], DT, tag="t2", name="t2", bufs=2)
                        for _ in range(2)]
            for t_ in t2_tiles:
                nc.vector.memset(t_, 1.0)
            hv_tiles = {}
            for d in "fb":
                for ph in range(2):
                    hv_tiles[d, ph] = [
                        hp.tile([128, L], DT, tag=f"h{d}{ph}", name=f"h{d}{ph}")
                        for _ in range(hp_bufs)]
                    for t_ in hv_tiles[d, ph]:
                        nc.vector.memset(
                            t_[:, 0:1] if d == "f" else t_[:, L - 1:L], 0.0)

            for r in range(repeats):
                def emit_head(bi, rr, hv):
                    # k-tile-major, dependency-light first: the rr k-tile has
                    # no scan dependency, then units in scan-completion order,
                    # so only the last k-tile waits on the final scan
                    ps = pp.tile([128, PSC], F32, tag="ps", name="ps")
                    ktiles = [(s_g1te, rr),
                              (s_g1[0], hv["f"][0]), (s_g1[1], hv["f"][1]),
                              (s_g1[2], hv["b"][0]), (s_g1[3], hv["b"][1])]
                    for ki, (w, rhs) in enumerate(ktiles):
                        for ch in range(PSC // FCH):
                            c0 = ch * FCH
                            nc.tensor.matmul(
                                ps[:, c0:c0 + FCH], lhsT=w,
                                rhs=rhs[:, c0:c0 + FCH],
                                start=(ki == 0), stop=(ki == len(ktiles) - 1))
                    rt_s = mp.tile([HH, L], DT, tag="rt", name="rt", bufs=2)
                    nc.scalar.activation(out=rt_s, in_=ps, func=AF.Relu,
                                         bias=s_gb1)
                    # out row = rt @ g2 (bias b2 added on host)
                    ps_o = pp.tile([1, PSC], F32, tag="ps", name="ps")
                    for ch in range(PSC // FCH):
                        c0 = ch * FCH
                        nc.tensor.matmul(ps_o[:, c0:c0 + FCH], lhsT=s_g2,
                                         rhs=rt_s[:, c0:c0 + FCH],
                                         start=True, stop=True)
                    orow = mp.tile([1, L], F32, tag="orow", name="orow", bufs=2)
                    nc.scalar.activation(out=orow, in_=ps_o, func=AF.Copy)
                    nc.sync.dma_start(out=d_out[bi:bi + 1, :], in_=orow)

                pending = None
                for bi in range(bs):
                    first = r == 0 and bi < 2

                    # ---- stage 1: rr = [relu(w1*t+b1)(64); x(2); ones] ----
                    t2 = t2_tiles[bi % 2]
                    nc.sync.dma_start(out=t2[0:1, :], in_=d_t[bi:bi + 1, :])
                    rr = rr_tiles[bi % 2]
                    nc.gpsimd.dma_start(out=rr[TE:TE + 2, :], in_=d_xT[bi])
                    ps_te = pp.tile([128, PSC], F32, tag="ps", name="ps")
                    for ch in range(PSC // FCH):
                        c0 = ch * FCH
                        nc.tensor.matmul(ps_te[0:TE, c0:c0 + FCH], lhsT=s_tw,
                                         rhs=t2[:, c0:c0 + FCH],
                                         start=True, stop=True)
                    nc.scalar.activation(out=rr[0:TE, :], in_=ps_te[0:TE, :],
                                         func=AF.Relu)

                    # ---- stage 2: gates + scans, 4 units ----
                    hv_out = {}
                    for di, d in enumerate("fb"):
                        hs = []
                        for ph in range(2):
                            uidx = bi * 4 + di * 2 + ph
                            ps_z = pp.tile([128, PSC], F32, tag="ps", name="ps")
                            for ch in range(PSC // FCH):
                                c0 = ch * FCH
                                nc.tensor.matmul(
                                    ps_z[:, c0:c0 + FCH],
                                    lhsT=s_wz[d][ph],
                                    rhs=rr[:, c0:c0 + FCH],
                                    start=True, stop=True)
                            ps_h = pp.tile([128, PSC], F32, tag="ps", name="ps")
                            for ch in range(PSC // FCH):
                                c0 = ch * FCH
                                nc.tensor.matmul(
                                    ps_h[:, c0:c0 + FCH],
                                    lhsT=s_wh[d][ph],
                                    rhs=rr[:, c0:c0 + FCH],
                                    start=True, stop=True)
                            zt = mp.tile([128, L], DT, tag="zt", name="zt")
                            nc.scalar.activation(out=zt, in_=ps_z,
                                                 func=AF.Sigmoid,
                                                 bias=s_bz[d][ph])
                            ht = mp.tile([128, L], DT, tag="ht", name="ht")
                            nc.scalar.activation(out=ht, in_=ps_h,
                                                 func=AF.Tanh,
                                                 bias=s_bh[d][ph])
                            # b = z*h~ (Pool, concurrent with DVE; a few on DVE)
                            bt = mp.tile([128, L], DT, tag="bt", name="bt")
                            beng = nc.vector if uidx in b_dve_idx else nc.gpsimd
                            beng.tensor_tensor(out=bt, in0=zt, in1=ht,
                                               op=ALU.mult)
                            # a = 1-z (DVE 4x)
                            at = mp.tile([128, L], DT, tag="at", name="at")
                            nc.vector.tensor_scalar(out=at, in0=zt,
                                                    scalar1=-1.0, scalar2=1.0,
                                                    op0=ALU.mult, op1=ALU.add)
                            # scan (shifted: reference stores pre-update state)
                            hv = hv_tiles[d, ph][(r * bs + bi) % hp_bufs]
                            if d == "f":
                                nc.vector.tensor_tensor_scan(
                                    out=hv[:, 1:L], data0=at[:, 0:L - 1],
                                    data1=bt[:, 0:L - 1], initial=0.0,
                                    op0=ALU.mult, op1=ALU.add)
                            else:
                                nc.vector.tensor_tensor_scan(
                                    out=_rev(hv, 0, L - 1),
                                    data0=_rev(at, 1, L - 1),
                                    data1=_rev(bt, 1, L - 1), initial=0.0,
                                    op0=ALU.mult, op1=ALU.add)
                            hs.append(hv)
                        hv_out[d] = hs

                    if pending is not None:
                        emit_head(*pending)
                    pending = (bi, rr, hv_out)
                emit_head(*pending)

    _spill_excess_waits(nc)
    return nc


def _host_prep(inputs):
    """Per-core input maps. The input projection and time-encoder second layer
    are composed into the gate/head weights (fp64) so the device operates
    directly on rr = [te1_hidden(64); x(2); ones(1)]."""
    f = {k: np.asarray(v, np.float64) for k, v in inputs.items()}

    def dt(a):
        return np.ascontiguousarray(a.astype(np.float32).astype(NP_DT))

    def f32c(a):
        return np.ascontiguousarray(a.astype(np.float32))

    def gate_w(pw, pb, w):
        """(67,256) weight in the rr basis for pre = (xc@[pw;pb]) @ w."""
        te_part = f["te_w2"] @ pw[2:66] @ w              # (64,256)
        x_part = pw[0:2] @ w                             # (2,256)
        ones_row = f["te_b2"] @ pw[2:66] @ w + pb @ w    # (256,)
        return np.concatenate([te_part, x_part, ones_row[None, :]], axis=0)

    common = {}
    gw = np.concatenate(
        [gate_w(f["fproj_w"], f["fproj_b"], f["fwz_w"]),
         gate_w(f["fproj_w"], f["fproj_b"], f["fwh_w"]),
         gate_w(f["bproj_w"], f["bproj_b"], f["bwz_w"]),
         gate_w(f["bproj_w"], f["bproj_b"], f["bwh_w"])], axis=1)   # (67, 1024)
    common["gw"] = dt(gw)
    bias = np.stack([f["fwz_b"][0:128], f["fwz_b"][128:256],
                     f["fwh_b"][0:128], f["fwh_b"][128:256],
                     f["bwz_b"][0:128], f["bwz_b"][128:256],
                     f["bwh_b"][0:128], f["bwh_b"][128:256],
                     f["gh_b1"]], axis=1)                           # (128, 9)
    common["bias"] = f32c(bias)
    g1blk = f["gh_w1"][0:2 * H].reshape(4, 128, HH)
    common["g1"] = dt(np.concatenate(
        [np.concatenate([g1blk[j] for j in range(4)], axis=1),
         f["gh_w2"]], axis=1))
    g1te = f["gh_w1"][2 * H:2 * H + TE]                  # (64,128)
    common["g1te"] = dt(np.concatenate(
        [f["te_w2"] @ g1te, np.zeros((2, HH)), (f["te_b2"] @ g1te)[None, :]], axis=0))
    common["tw"] = dt(np.concatenate([f["te_w1"].reshape(1, TE),
                                      f["te_b1"].reshape(1, TE)], axis=0))
    in_maps = []
    for c in range(NCORES):
        sl = slice(BS * c, BS * (c + 1))
        m = dict(common)
        m["xT"] = dt(f["x"][sl].transpose(0, 2, 1))
        m["t"] = dt(f["t"][sl, :, 0])
        in_maps.append(m)
    return in_maps, float(f["gh_b2"][0])


_CACHE = {}


def _get_nc():
    if "nc" not in _CACHE:
        _CACHE["nc"] = _build_nc()
    return _CACHE["nc"]


def kernel(**inputs):
    nc = _get_nc()
    in_maps, gh_b2 = _host_prep(inputs)
    res = run_bass_kernel_spmd(nc, in_maps, list(range(NCORES)))
    out = np.empty((B, L, 1), np.float32)
    for c in range(NCORES):
        out[BS * c:BS * (c + 1), :, 0] = res.results[c]["out"] + gh_b2
    return out


def _build_sharded_exec(nc):
    """Non-donating clone of bass2jax.run_bass_via_pjrt's multi-core path so
    the executable can be launched repeatedly for timing."""
    import jax
    import concourse.mybir as mb
    from jax.experimental.shard_map import shard_map
    from jax.sharding import Mesh, PartitionSpec
    from concourse import bass2jax

    bass2jax.install_neuronx_cc_hook()
    part_name = nc.partition_id_tensor.name if nc.partition_id_tensor else None
    in_names, out_names, out_avals, zero_outs = [], [], [], []
    for alloc in nc.m.functions[0].allocations:
        if not isinstance(alloc, mb.MemoryLocationSet):
            continue
        name = alloc.memorylocations[0].name
        if alloc.kind == "ExternalInput":
            if name != part_name:
                in_names.append(name)
        elif alloc.kind == "ExternalOutput":
            shape = tuple(alloc.tensor_shape)
            dtype = mb.dt.np(alloc.dtype)
            out_names.append(name)
            out_avals.append(jax.core.ShapedArray(shape, dtype))
            zero_outs.append(np.zeros(shape, dtype))
    n_params = len(in_names)
    all_names = in_names + out_names
    if part_name is not None:
        all_names = all_names + [part_name]

    def _body(*args):
        operands = list(args)
        if part_name is not None:
            operands.append(bass2jax.partition_id_tensor())
        outs = bass2jax._bass_exec_p.bind(
            *operands,
            out_avals=tuple(out_avals),
            in_names=tuple(all_names),
            out_names=tuple(out_names),
            lowering_input_output_aliases=(),
            sim_require_finite=True,
            sim_require_nnan=True,
            nc=nc,
        )
        return tuple(outs)

    devices = jax.devices()[:NCORES]
    mesh = Mesh(np.asarray(devices), ("core",))
    nin = n_params + len(out_names)
    sharded = jax.jit(
        shard_map(_body, mesh=mesh,
                  in_specs=(PartitionSpec("core"),) * nin,
                  out_specs=(PartitionSpec("core"),) * len(out_names),
                  check_rep=False),
        keep_unused=True,
    )
    return sharded, mesh, in_names, out_names, zero_outs


def _timed_launch(nc, in_maps, iters):
    import jax
    from jax.sharding import NamedSharding, PartitionSpec

    sharded, mesh, in_names, out_names, zero_outs = _build_sharded_exec(nc)
    sh = NamedSharding(mesh, PartitionSpec("core"))
    concat_in = [
        np.concatenate([np.asarray(in_maps[c][n]) for c in range(NCORES)], axis=0)
        for n in in_names
    ]
    concat_zero = [
        np.zeros((NCORES * z.shape[0], *z.shape[1:]), z.dtype) for z in zero_outs
    ]
    args = [jax.device_put(a, sh) for a in concat_in + concat_zero]
    out = sharded(*args)
    jax.block_until_ready(out)
    ts = []
    for _ in range(iters):
        t0 = time.perf_counter()
        out = sharded(*args)
        jax.block_until_ready(out)
        ts.append(time.perf_counter() - t0)
    return min(ts)


def bench(inputs, iters=10, r_hi=5):
    """Estimate on-device kernel time (ns) free of launch overhead: build the
    same kernel with the per-core work repeated 1x and r_hi x inside one NEFF
    and report the slope ((t_hi - t_1) / (r_hi - 1))."""
    in_maps, _ = _host_prep(inputs)
    t1 = _timed_launch(_build_nc(repeats=1), in_maps, iters)
    th = _timed_launch(_build_nc(repeats=r_hi), in_maps, iters)
    print(f"bench: launch r=1 {t1*1e6:.0f} us, r={r_hi} {th*1e6:.0f} us")
    return (th - t1) / (r_hi - 1) * 1e9
